# revision 15
# baseline (speedup 1.0000x reference)
"""Blocksparse dilated attention TRN2 kernel.

Sharding: 8 cores = r(=4 dilation offsets) x B(=2 batch). Each core runs one
independent per-offset attention branch on its strided token subset
(x[b, o::r, :]), with that offset's own weights. Host does the strided
gather (+transpose to channel-major) and the final scatter into the
zero-padded (B, S, r*D) output.

Per-core math (L=2048 tokens, D=768, H=12 heads, hd=64, segment=512):
  qkvT = Wqkv @ xoT            (channel-on-partition for q,k; token-major v)
  per (segment, head):  scoresT = kT-chunks.T x qT   (k on partitions)
                        attnT = exp(scale * scoresT)  (no max-subtract:
                              scores std ~0.3, max ~1.5 -> exp safe in fp32)
                        [ctxu; denom] = [v | ones].T @ attnT  (ones column
                              makes PSUM row 64 the softmax denominator)
  per segment (batched over heads, off the PE critical path):
                        rc = 1/denom  (one DVE reciprocal for all 12 heads;
                              per-head reciprocals measured 3.35us each and
                              stalled the PE into HAM re-throttle)
                        ctxT = ctxu * broadcast(rc)  (broadcast via PE
                              outer-product matmul ones(1,64).T @ rc_row)
  outT = Wout @ ctxT + bout

Matmuls run in bf16 (full PE rate; fp32 PSUM accumulation). fp32/fp32r
matmuls are unusable here: walrus fuses the weight load into the matmul and
the fused instruction has a single sync-wait slot ("Too many sync wait
commands" at codegen for any matmul with two cross-engine dependencies).

Emission order software-pipelines segments so the PE never idles long
enough (~3.4us) for the HAM clock gate to drop it from 2.4 to 1.2 GHz:
  proj(s) -> [normalize(s-1) + outproj(s-1)] -> attention(s)
"""

import math
import sys
from contextlib import ExitStack

import ml_dtypes
import numpy as np

for _p in ("/opt/trn_rl_repo",):
    if _p not in sys.path:
        sys.path.insert(0, _p)

import concourse.bass as bass
import concourse.mybir as mybir
import concourse.tile as tile
from concourse import bacc
from concourse.bass_utils import run_bass_kernel_spmd

P = 128

# Problem constants (hardcoded per harness contract)
B0, S0, D0 = 2, 8192, 768
R0 = 4
H0, HD0 = 12, 64
SEG0 = 512
NSEG0 = (S0 // R0) // SEG0  # 4
N_CORES = 8

F32 = mybir.dt.float32
F32R = mybir.dt.float32r
BF16 = mybir.dt.bfloat16


def build_nc(D=D0, H=H0, HD=HD0, SEG=SEG0, NSEG=NSEG0, mm_dt=BF16):
    """Build the per-core Bass program (same NEFF on all cores)."""
    DC = D // P                # channel chunks of 128
    L = SEG * NSEG             # tokens per core
    KC = SEG // P              # key chunks per segment
    HPC = P // HD              # heads per 128-channel chunk
    E3 = 3 * D
    HV = HD + 1                # v columns per head incl. ones column
    scale = 1.0 / math.sqrt(HD)
    assert D == H * HD and SEG % P == 0 and D % P == 0 and KC % 2 == 0

    nc = bacc.Bacc(trn_type="TRN2")
    xoT = nc.dram_tensor("xoT", [D, L], mm_dt, kind="ExternalInput")
    wqkvT = nc.dram_tensor("wqkvT", [D, E3], mm_dt, kind="ExternalInput")
    woutT = nc.dram_tensor("woutT", [D, D], mm_dt, kind="ExternalInput")
    bqkv_pt = nc.dram_tensor("bqkv_pt", [P, 3 * DC], F32, kind="ExternalInput")
    bout_pt = nc.dram_tensor("bout_pt", [P, DC], F32, kind="ExternalInput")
    bv = nc.dram_tensor("bv", [D], F32, kind="ExternalInput")
    outT = nc.dram_tensor("outT", [D, L], F32, kind="ExternalOutput")

    with ExitStack() as ctx:
        tc = ctx.enter_context(tile.TileContext(nc))
        singles = ctx.enter_context(tc.tile_pool(name="singles", bufs=1))
        xo_pool = ctx.enter_context(tc.tile_pool(name="xo", bufs=2))
        qk_pool = ctx.enter_context(tc.tile_pool(name="qk", bufs=1))
        v_pool = ctx.enter_context(tc.tile_pool(name="v", bufs=1))
        attn_pool = ctx.enter_context(tc.tile_pool(name="attn", bufs=2))
        ctxu_pool = ctx.enter_context(tc.tile_pool(name="ctxu", bufs=2))
        den_pool = ctx.enter_context(tc.tile_pool(name="den", bufs=2))
        ctxs_pool = ctx.enter_context(tc.tile_pool(name="ctxs", bufs=1))
        out_pool = ctx.enter_context(tc.tile_pool(name="outp", bufs=3))
        pp_proj = ctx.enter_context(tc.tile_pool(name="pp_proj", bufs=2, space="PSUM"))
        pp_scA = ctx.enter_context(tc.tile_pool(name="pp_scA", bufs=1, space="PSUM"))
        pp_scB = ctx.enter_context(tc.tile_pool(name="pp_scB", bufs=1, space="PSUM"))
        pp_cb = ctx.enter_context(tc.tile_pool(name="pp_cb", bufs=2, space="PSUM"))

        # --- segment-0 input first: it gates the very first matmul group ---
        xo_first = xo_pool.tile([P, DC, SEG], mm_dt, tag="xo", name="xo_s0")
        for dc in range(DC):
            nc.sync.dma_start(out=xo_first[:, dc, :], in_=xoT[dc * P:(dc + 1) * P, 0:SEG])

        # --- persistent weights / biases, in first-use order (q sections gate
        # the first matmuls; k needed ~20us in; v after that; wout last) ---
        w_qkv_sb = singles.tile([P, DC, E3], mm_dt, tag="wqkv")
        for sec in range(3):
            for dc in range(DC):
                nc.sync.dma_start(
                    out=w_qkv_sb[:, dc, sec * D:(sec + 1) * D],
                    in_=wqkvT[dc * P:(dc + 1) * P, sec * D:(sec + 1) * D])
        bqkv_sb = singles.tile([P, 3 * DC], F32, tag="bqkv")
        nc.sync.dma_start(out=bqkv_sb, in_=bqkv_pt[:, :])
        bout_sb = singles.tile([P, DC], F32, tag="bout")
        nc.sync.dma_start(out=bout_sb, in_=bout_pt[:, :])
        # v-section bias broadcast along partitions (natural layout add)
        bv_sb = singles.tile([P, D], F32, tag="bv")
        bv_ap = bv[:]
        bv_bcast = bass.AP(tensor=bv_ap.tensor, offset=bv_ap.offset,
                           ap=[[0, P], *bv_ap.ap])
        nc.gpsimd.dma_start(out=bv_sb, in_=bv_bcast)
        w_out_sb = singles.tile([P, DC, D], mm_dt, tag="wout")
        for dc in range(DC):
            nc.sync.dma_start(out=w_out_sb[:, dc, :], in_=woutT[dc * P:(dc + 1) * P, :])
        ones_sb = singles.tile([1, HD], mm_dt, tag="ones")
        nc.vector.memset(ones_sb, 1.0)

        def load_and_proj(s):
            """xo load + qkv projections for segment s."""
            st = {}
            if s == 0:
                xo_s = xo_first
            else:
                xo_s = xo_pool.tile([P, DC, SEG], mm_dt, tag="xo", name=f"xo_s{s}")
                for dc in range(DC):
                    nc.sync.dma_start(
                        out=xo_s[:, dc, :],
                        in_=xoT[dc * P:(dc + 1) * P, s * SEG:(s + 1) * SEG])
            st["xo"] = xo_s

            # q,k in transposed layout (e on partitions)
            qk_s = qk_pool.tile([P, 2 * DC, SEG], mm_dt, tag="qk", name=f"qk_s{s}")
            st["qk"] = qk_s
            for ec in range(2 * DC):
                ps = pp_proj.tile([P, SEG], F32, tag="proj", name=f"psqk{s}_{ec}")
                for dc in range(DC):
                    nc.tensor.matmul(
                        ps,
                        w_qkv_sb[:, dc, ec * P:(ec + 1) * P],
                        xo_s[:, dc, :],
                        start=(dc == 0), stop=(dc == DC - 1))
                nc.vector.tensor_scalar_add(qk_s[:, ec, :], ps, bqkv_sb[:, ec:ec + 1])

            # v in natural layout (token on partitions), per-head + ones column
            v_s = v_pool.tile([P, KC, H * HV], mm_dt, tag="v", name=f"v_s{s}")
            st["v"] = v_s
            v_view = v_s.rearrange("p k (h c) -> p k h c", c=HV)
            nc.vector.memset(v_view[:, :, :, HD:HD + 1], 1.0)
            for lc in range(KC):
                for n0 in range(0, D, 512):
                    n = min(512, D - n0)
                    nh = n // HD
                    h0 = n0 // HD
                    psv = pp_proj.tile([P, SEG], F32, tag="proj",
                                       name=f"psv{s}_{lc}_{n0}")
                    for dc in range(DC):
                        nc.tensor.matmul(
                            psv[:, :n],
                            xo_s[:, dc, lc * P:(lc + 1) * P],
                            w_qkv_sb[:, dc, 2 * D + n0: 2 * D + n0 + n],
                            start=(dc == 0), stop=(dc == DC - 1))
                    nc.vector.tensor_add(
                        v_view[:, lc, h0:h0 + nh, 0:HD],
                        psv[:, :n].rearrange("p (h c) -> p h c", c=HD),
                        bv_sb[:, n0:n0 + n].rearrange("p (h c) -> p h c", c=HD))
            return st

        def attention(s, st, filler=()):
            """scores + exp + unnormalized ctx (and denom) per head,
            software-pipelined: scores(h) run while exp(h-1) finishes.
            `filler` tasks (prev segment's normalize + outproj) are emitted
            between heads so the PE has work while ACT exp catches up."""
            filler = list(filler)
            n_filler = len(filler)
            emitted = 0
            qk_s, v_s = st["qk"], st["v"]
            ctxu = ctxu_pool.tile([P, DC, SEG], F32, tag="ctxu", name=f"ctxu{s}")
            den = den_pool.tile([1, H * SEG], F32, tag="den", name=f"den{s}")
            st["ctxu"], st["den"] = ctxu, den
            ats = {}
            for h in range(H + 1):
                while emitted < (h * n_filler) // H:
                    filler[emitted]()
                    emitted += 1
                if h < H:
                    hc = h // HPC
                    ho = (h % HPC) * HD
                    at = attn_pool.tile([P, KC, SEG], mm_dt, tag="attn",
                                        name=f"at{s}_{h}")
                    ats[h] = at
                    for half, pool in ((0, pp_scA), (1, pp_scB)):
                        sc = pool.tile([P, KC // 2, SEG], F32,
                                       tag=f"sc{half}", name=f"sc{half}_{s}_{h}")
                        for j in range(KC // 2):
                            kc = half * (KC // 2) + j
                            nc.tensor.matmul(
                                sc[:, j, :],
                                qk_s[ho:ho + HD, DC + hc, kc * P:(kc + 1) * P],
                                qk_s[ho:ho + HD, hc, :])
                        nc.scalar.activation(
                            at[:, half * (KC // 2):(half + 1) * (KC // 2), :],
                            sc,
                            mybir.ActivationFunctionType.Exp,
                            scale=scale)
                if h > 0:
                    hp = h - 1
                    hc = hp // HPC
                    ho = (hp % HPC) * HD
                    at = ats.pop(hp)
                    cps = pp_cb.tile([HD + 1, SEG], F32, tag="cb",
                                     name=f"cps{s}_{hp}")
                    for kc in range(KC):
                        nc.tensor.matmul(
                            cps,
                            v_s[:, kc, hp * HV:(hp + 1) * HV],
                            at[:, kc, :],
                            start=(kc == 0), stop=(kc == KC - 1))
                    nc.vector.tensor_copy(ctxu[ho:ho + HD, hc, :], cps[0:HD, :])
                    nc.vector.tensor_copy(den[0:1, hp * SEG:(hp + 1) * SEG],
                                          cps[HD:HD + 1, :])

        def norm_tasks(s, st):
            """Emit the reciprocal chain now (DMA/DVE only); return deferred
            PE tasks: 12x [broadcast + normalize-mul], then 6x outproj group.
            Order matters: outproj reads every head's normalized ctx."""
            ctxu, den = st["ctxu"], st["den"]
            # DVE reciprocal costs ~6.5ns/element/lane, so a single-partition
            # strip would take ~40us. Round-trip a DMA "transpose" to spread
            # the elements over all 128 partitions (element order irrelevant:
            # reciprocal is elementwise and the second DMA restores order).
            assert (H * SEG) % P == 0
            den_t = den_pool.tile([P, H * SEG // P], F32, tag="dent",
                                  name=f"dent{s}")
            nc.sync.dma_start(out=den_t, in_=den[0:1, :])
            rc_t = den_pool.tile([P, H * SEG // P], mm_dt, tag="rct",
                                 name=f"rct{s}")
            with nc.allow_low_precision(
                    reason="softmax denominator reciprocal; bf16 scale factor"):
                nc.vector.reciprocal(rc_t, den_t)
            rc = den_pool.tile([1, H * SEG], mm_dt, tag="rc", name=f"rc{s}")
            nc.sync.dma_start(out=rc, in_=rc_t)
            ctx_s = ctxs_pool.tile([P, DC, SEG], mm_dt, tag="ctxs", name=f"cs{s}")

            def norm_head(h):
                hc = h // HPC
                ho = (h % HPC) * HD
                bc = pp_cb.tile([HD, SEG], F32, tag="cb", name=f"bc{s}_{h}")
                nc.tensor.matmul(bc, ones_sb,
                                 rc[0:1, h * SEG:(h + 1) * SEG])
                nc.vector.tensor_mul(ctx_s[ho:ho + HD, hc, :],
                                     ctxu[ho:ho + HD, hc, :], bc)

            def outproj(fc):
                pso = pp_proj.tile([P, SEG], F32, tag="proj", name=f"pso{s}_{fc}")
                for dc in range(DC):
                    nc.tensor.matmul(
                        pso,
                        w_out_sb[:, dc, fc * P:(fc + 1) * P],
                        ctx_s[:, dc, :],
                        start=(dc == 0), stop=(dc == DC - 1))
                ot = out_pool.tile([P, SEG], F32, tag="ot", name=f"ot{s}_{fc}")
                nc.vector.tensor_scalar_add(ot, pso, bout_sb[:, fc:fc + 1])
                nc.sync.dma_start(
                    out=outT[fc * P:(fc + 1) * P, s * SEG:(s + 1) * SEG], in_=ot)

            return ([(lambda h=h: norm_head(h)) for h in range(H)]
                    + [(lambda fc=fc: outproj(fc)) for fc in range(DC)])

        sts = {}
        for s in range(NSEG):
            sts[s] = load_and_proj(s)
            filler = norm_tasks(s - 1, sts.pop(s - 1)) if s > 0 else ()
            attention(s, sts[s], filler)
        for task in norm_tasks(NSEG - 1, sts.pop(NSEG - 1)):
            task()

    nc.compile()
    return nc


def make_in_maps(x, Wqkv, bqkv, Wout, bout):
    """Shard full inputs across 8 cores: core = o*B + b."""
    r, E3, D = Wqkv.shape
    Bb, S, _ = x.shape
    DC = D // P
    in_maps = []
    for c in range(r * Bb):
        o, b = c // Bb, c % Bb
        in_maps.append({
            "xoT": np.ascontiguousarray(x[b, o::r, :].T).astype(ml_dtypes.bfloat16),
            "wqkvT": np.ascontiguousarray(Wqkv[o].T).astype(ml_dtypes.bfloat16),
            "woutT": np.ascontiguousarray(Wout[o].T).astype(ml_dtypes.bfloat16),
            "bqkv_pt": np.ascontiguousarray(bqkv[o].reshape(3 * DC, P).T),
            "bout_pt": np.ascontiguousarray(bout[o].reshape(DC, P).T),
            "bv": np.ascontiguousarray(bqkv[o, 2 * D:3 * D]),
        })
    return in_maps


_NC_CACHE = {}


def get_nc():
    if "nc" not in _NC_CACHE:
        _NC_CACHE["nc"] = build_nc()
    return _NC_CACHE["nc"]


def run(inputs, trace=False, **kwargs):
    """Run the SPMD kernel; returns (full_output, BassKernelResults)."""
    x = np.ascontiguousarray(np.asarray(inputs["x"], dtype=np.float32))
    Wqkv = np.asarray(inputs["Wqkv"], dtype=np.float32)
    bqkv = np.asarray(inputs["bqkv"], dtype=np.float32)
    Wout = np.asarray(inputs["Wout"], dtype=np.float32)
    bout = np.asarray(inputs["bout"], dtype=np.float32)
    r, E3, D = Wqkv.shape
    Bb, S, _ = x.shape

    nc = get_nc()
    in_maps = make_in_maps(x, Wqkv, bqkv, Wout, bout)
    res = run_bass_kernel_spmd(nc, in_maps, core_ids=list(range(len(in_maps))),
                               trace=trace, **kwargs)

    out = np.zeros((Bb, S, r * D), np.float32)
    for c in range(len(in_maps)):
        o, b = c // Bb, c % Bb
        out[b, o::r, o * D:(o + 1) * D] = res.results[c]["outT"].T
    return out, res


def kernel(x, Wqkv, bqkv, Wout, bout, num_heads):
    assert int(num_heads) == H0
    out, _ = run(dict(x=x, Wqkv=Wqkv, bqkv=bqkv, Wout=Wout, bout=bout))
    return out


# revision 16
# speedup vs baseline: 1.2065x; 1.2065x over previous
"""Blocksparse dilated attention TRN2 kernel.

Sharding: 8 cores = r(=4 dilation offsets) x B(=2 batch). Each core runs one
independent per-offset attention branch on its strided token subset
(x[b, o::r, :]), with that offset's own weights. Host does the strided
gather (+transpose to channel-major) and the final scatter into the
zero-padded (B, S, r*D) output.

Per-core math (L=2048 tokens, D=768, H=12 heads, hd=64, segment=512):
  qkvT = Wqkv @ xoT            (channel-on-partition for q,k; token-major v)
  per (segment, head):  scoresT = kT-chunks.T x qT   (k on partitions)
                        attnT = exp(scale * scoresT)  (no max-subtract:
                              scores std ~0.3, max ~1.5 -> exp safe in fp32)
                        [ctxu; denom] = [v | ones].T @ attnT  (ones column
                              makes PSUM row 64 the softmax denominator)
  per segment (batched over heads, off the PE critical path):
                        rc = 1/denom  (one DVE reciprocal for all 12 heads;
                              per-head reciprocals measured 3.35us each and
                              stalled the PE into HAM re-throttle)
                        ctxT = ctxu * broadcast(rc)  (broadcast via PE
                              outer-product matmul ones(1,64).T @ rc_row)
  outT = Wout @ ctxT + bout

Matmuls run in bf16 (full PE rate; fp32 PSUM accumulation). fp32/fp32r
matmuls are unusable here: walrus fuses the weight load into the matmul and
the fused instruction has a single sync-wait slot ("Too many sync wait
commands" at codegen for any matmul with two cross-engine dependencies).

Emission order software-pipelines segments so the PE never idles long
enough (~3.4us) for the HAM clock gate to drop it from 2.4 to 1.2 GHz:
  proj(s) -> [normalize(s-1) + outproj(s-1)] -> attention(s)
"""

import math
import sys
from contextlib import ExitStack

import ml_dtypes
import numpy as np

for _p in ("/opt/trn_rl_repo",):
    if _p not in sys.path:
        sys.path.insert(0, _p)

import concourse.bass as bass
import concourse.mybir as mybir
import concourse.tile as tile
from concourse import bacc
from concourse.bass_utils import run_bass_kernel_spmd

P = 128

# Problem constants (hardcoded per harness contract)
B0, S0, D0 = 2, 8192, 768
R0 = 4
H0, HD0 = 12, 64
SEG0 = 512
NSEG0 = (S0 // R0) // SEG0  # 4
N_CORES = 8

F32 = mybir.dt.float32
F32R = mybir.dt.float32r
BF16 = mybir.dt.bfloat16


def build_nc(D=D0, H=H0, HD=HD0, SEG=SEG0, NSEG=NSEG0, mm_dt=BF16):
    """Build the per-core Bass program (same NEFF on all cores)."""
    DC = D // P                # channel chunks of 128
    L = SEG * NSEG             # tokens per core
    KC = SEG // P              # key chunks per segment
    HPC = P // HD              # heads per 128-channel chunk
    E3 = 3 * D
    HV = HD + 1                # v columns per head incl. ones column
    scale = 1.0 / math.sqrt(HD)
    assert D == H * HD and SEG % P == 0 and D % P == 0 and KC % 2 == 0

    nc = bacc.Bacc(trn_type="TRN2")
    xoT = nc.dram_tensor("xoT", [D, L], mm_dt, kind="ExternalInput")
    wqkvT = nc.dram_tensor("wqkvT", [D, E3], mm_dt, kind="ExternalInput")
    woutT = nc.dram_tensor("woutT", [D, D], mm_dt, kind="ExternalInput")
    bqkv_pt = nc.dram_tensor("bqkv_pt", [P, 3 * DC], F32, kind="ExternalInput")
    bout_pt = nc.dram_tensor("bout_pt", [P, DC], F32, kind="ExternalInput")
    bv = nc.dram_tensor("bv", [D], F32, kind="ExternalInput")
    outT = nc.dram_tensor("outT", [D, L], F32, kind="ExternalOutput")

    with ExitStack() as ctx:
        tc = ctx.enter_context(tile.TileContext(nc))
        singles = ctx.enter_context(tc.tile_pool(name="singles", bufs=1))
        xo_pool = ctx.enter_context(tc.tile_pool(name="xo", bufs=2))
        qk_pool = ctx.enter_context(tc.tile_pool(name="qk", bufs=1))
        v_pool = ctx.enter_context(tc.tile_pool(name="v", bufs=1))
        attn_pool = ctx.enter_context(tc.tile_pool(name="attn", bufs=2))
        ctxu_pool = ctx.enter_context(tc.tile_pool(name="ctxu", bufs=2))
        den_pool = ctx.enter_context(tc.tile_pool(name="den", bufs=2))
        ctxs_pool = ctx.enter_context(tc.tile_pool(name="ctxs", bufs=1))
        out_pool = ctx.enter_context(tc.tile_pool(name="outp", bufs=3))
        pp_proj = ctx.enter_context(tc.tile_pool(name="pp_proj", bufs=2, space="PSUM"))
        pp_scA = ctx.enter_context(tc.tile_pool(name="pp_scA", bufs=1, space="PSUM"))
        pp_scB = ctx.enter_context(tc.tile_pool(name="pp_scB", bufs=1, space="PSUM"))
        pp_cb = ctx.enter_context(tc.tile_pool(name="pp_cb", bufs=2, space="PSUM"))

        # --- segment-0 input first: it gates the very first matmul group ---
        xo_first = xo_pool.tile([P, DC, SEG], mm_dt, tag="xo", name="xo_s0")
        for dc in range(DC):
            nc.sync.dma_start(out=xo_first[:, dc, :], in_=xoT[dc * P:(dc + 1) * P, 0:SEG])

        # --- persistent weights / biases, in first-use order: tiny biases
        # first (a late bias DMA stalls the first qk-add on DVE and cascades
        # into a PE psum-WAR stall), then q sections (gate the first
        # matmuls), k, v, wout last ---
        bqkv_sb = singles.tile([P, 3 * DC], F32, tag="bqkv")
        nc.sync.dma_start(out=bqkv_sb, in_=bqkv_pt[:, :])
        bout_sb = singles.tile([P, DC], F32, tag="bout")
        nc.sync.dma_start(out=bout_sb, in_=bout_pt[:, :])
        # v-section bias broadcast along partitions (natural layout add)
        bv_sb = singles.tile([P, D], F32, tag="bv")
        bv_ap = bv[:]
        bv_bcast = bass.AP(tensor=bv_ap.tensor, offset=bv_ap.offset,
                           ap=[[0, P], *bv_ap.ap])
        nc.gpsimd.dma_start(out=bv_sb, in_=bv_bcast)
        w_qkv_sb = singles.tile([P, DC, E3], mm_dt, tag="wqkv")
        for sec in range(3):
            for dc in range(DC):
                nc.sync.dma_start(
                    out=w_qkv_sb[:, dc, sec * D:(sec + 1) * D],
                    in_=wqkvT[dc * P:(dc + 1) * P, sec * D:(sec + 1) * D])
        w_out_sb = singles.tile([P, DC, D], mm_dt, tag="wout")
        for dc in range(DC):
            nc.sync.dma_start(out=w_out_sb[:, dc, :], in_=woutT[dc * P:(dc + 1) * P, :])
        ones_sb = singles.tile([1, HD], mm_dt, tag="ones")
        nc.vector.memset(ones_sb, 1.0)

        def load_and_proj(s):
            """xo load + qkv projections for segment s."""
            st = {}
            if s == 0:
                xo_s = xo_first
            else:
                xo_s = xo_pool.tile([P, DC, SEG], mm_dt, tag="xo", name=f"xo_s{s}")
                for dc in range(DC):
                    nc.sync.dma_start(
                        out=xo_s[:, dc, :],
                        in_=xoT[dc * P:(dc + 1) * P, s * SEG:(s + 1) * SEG])
            st["xo"] = xo_s

            # q,k in transposed layout (e on partitions)
            qk_s = qk_pool.tile([P, 2 * DC, SEG], mm_dt, tag="qk", name=f"qk_s{s}")
            st["qk"] = qk_s
            for ec in range(2 * DC):
                ps = pp_proj.tile([P, SEG], F32, tag="proj", name=f"psqk{s}_{ec}")
                for dc in range(DC):
                    nc.tensor.matmul(
                        ps,
                        w_qkv_sb[:, dc, ec * P:(ec + 1) * P],
                        xo_s[:, dc, :],
                        start=(dc == 0), stop=(dc == DC - 1))
                nc.vector.tensor_scalar_add(qk_s[:, ec, :], ps, bqkv_sb[:, ec:ec + 1])

            # v in natural layout (token on partitions), per-head + ones column
            v_s = v_pool.tile([P, KC, H * HV], mm_dt, tag="v", name=f"v_s{s}")
            st["v"] = v_s
            v_view = v_s.rearrange("p k (h c) -> p k h c", c=HV)
            nc.vector.memset(v_view[:, :, :, HD:HD + 1], 1.0)
            for lc in range(KC):
                for n0 in range(0, D, 512):
                    n = min(512, D - n0)
                    nh = n // HD
                    h0 = n0 // HD
                    psv = pp_proj.tile([P, SEG], F32, tag="proj",
                                       name=f"psv{s}_{lc}_{n0}")
                    for dc in range(DC):
                        nc.tensor.matmul(
                            psv[:, :n],
                            xo_s[:, dc, lc * P:(lc + 1) * P],
                            w_qkv_sb[:, dc, 2 * D + n0: 2 * D + n0 + n],
                            start=(dc == 0), stop=(dc == DC - 1))
                    nc.vector.tensor_add(
                        v_view[:, lc, h0:h0 + nh, 0:HD],
                        psv[:, :n].rearrange("p (h c) -> p h c", c=HD),
                        bv_sb[:, n0:n0 + n].rearrange("p (h c) -> p h c", c=HD))
            return st

        def attention(s, st, filler=()):
            """scores + exp + unnormalized ctx (and denom) per head,
            software-pipelined: scores(h) run while exp(h-1) finishes.
            `filler` tasks (prev segment's normalize + outproj) are emitted
            between heads so the PE has work while ACT exp catches up."""
            filler = list(filler)
            n_filler = len(filler)
            emitted = 0
            qk_s, v_s = st["qk"], st["v"]
            ctxu = ctxu_pool.tile([P, DC, SEG], F32, tag="ctxu", name=f"ctxu{s}")
            den = den_pool.tile([1, H * SEG], F32, tag="den", name=f"den{s}")
            st["ctxu"], st["den"] = ctxu, den
            ats = {}
            for h in range(H + 1):
                while emitted < (h * n_filler) // H:
                    filler[emitted]()
                    emitted += 1
                if h < H:
                    hc = h // HPC
                    ho = (h % HPC) * HD
                    at = attn_pool.tile([P, KC, SEG], mm_dt, tag="attn",
                                        name=f"at{s}_{h}")
                    ats[h] = at
                    for half, pool in ((0, pp_scA), (1, pp_scB)):
                        sc = pool.tile([P, KC // 2, SEG], F32,
                                       tag=f"sc{half}", name=f"sc{half}_{s}_{h}")
                        for j in range(KC // 2):
                            kc = half * (KC // 2) + j
                            nc.tensor.matmul(
                                sc[:, j, :],
                                qk_s[ho:ho + HD, DC + hc, kc * P:(kc + 1) * P],
                                qk_s[ho:ho + HD, hc, :])
                        nc.scalar.activation(
                            at[:, half * (KC // 2):(half + 1) * (KC // 2), :],
                            sc,
                            mybir.ActivationFunctionType.Exp,
                            scale=scale)
                if h > 0:
                    hp = h - 1
                    hc = hp // HPC
                    ho = (hp % HPC) * HD
                    at = ats.pop(hp)
                    cps = pp_cb.tile([HD + 1, SEG], F32, tag="cb",
                                     name=f"cps{s}_{hp}")
                    for kc in range(KC):
                        nc.tensor.matmul(
                            cps,
                            v_s[:, kc, hp * HV:(hp + 1) * HV],
                            at[:, kc, :],
                            start=(kc == 0), stop=(kc == KC - 1))
                    nc.vector.tensor_copy(ctxu[ho:ho + HD, hc, :], cps[0:HD, :])
                    nc.vector.tensor_copy(den[0:1, hp * SEG:(hp + 1) * SEG],
                                          cps[HD:HD + 1, :])

        def norm_tasks(s, st):
            """Emit the reciprocal chain now (DMA/DVE only); return deferred
            PE tasks: 12x [broadcast + normalize-mul], then 6x outproj group.
            Order matters: outproj reads every head's normalized ctx."""
            ctxu, den = st["ctxu"], st["den"]
            # DVE reciprocal costs ~6.5ns/element/lane, so a single-partition
            # strip would take ~40us. Round-trip a DMA "transpose" to spread
            # the elements over all 128 partitions (element order irrelevant:
            # reciprocal is elementwise and the second DMA restores order).
            assert (H * SEG) % P == 0
            den_t = den_pool.tile([P, H * SEG // P], F32, tag="dent",
                                  name=f"dent{s}")
            nc.gpsimd.dma_start(out=den_t, in_=den[0:1, :])
            rc_t = den_pool.tile([P, H * SEG // P], mm_dt, tag="rct",
                                 name=f"rct{s}")
            with nc.allow_low_precision(
                    reason="softmax denominator reciprocal; bf16 scale factor"):
                nc.vector.reciprocal(rc_t, den_t)
            rc = den_pool.tile([1, H * SEG], mm_dt, tag="rc", name=f"rc{s}")
            nc.gpsimd.dma_start(out=rc, in_=rc_t)
            ctx_s = ctxs_pool.tile([P, DC, SEG], mm_dt, tag="ctxs", name=f"cs{s}")

            def norm_head(h):
                hc = h // HPC
                ho = (h % HPC) * HD
                bc = pp_cb.tile([HD, SEG], F32, tag="cb", name=f"bc{s}_{h}")
                nc.tensor.matmul(bc, ones_sb,
                                 rc[0:1, h * SEG:(h + 1) * SEG])
                nc.vector.tensor_mul(ctx_s[ho:ho + HD, hc, :],
                                     ctxu[ho:ho + HD, hc, :], bc)

            def outproj(fc):
                pso = pp_proj.tile([P, SEG], F32, tag="proj", name=f"pso{s}_{fc}")
                for dc in range(DC):
                    nc.tensor.matmul(
                        pso,
                        w_out_sb[:, dc, fc * P:(fc + 1) * P],
                        ctx_s[:, dc, :],
                        start=(dc == 0), stop=(dc == DC - 1))
                ot = out_pool.tile([P, SEG], F32, tag="ot", name=f"ot{s}_{fc}")
                nc.vector.tensor_scalar_add(ot, pso, bout_sb[:, fc:fc + 1])
                nc.sync.dma_start(
                    out=outT[fc * P:(fc + 1) * P, s * SEG:(s + 1) * SEG], in_=ot)

            return ([(lambda h=h: norm_head(h)) for h in range(H)]
                    + [(lambda fc=fc: outproj(fc)) for fc in range(DC)])

        sts = {}
        for s in range(NSEG):
            sts[s] = load_and_proj(s)
            filler = norm_tasks(s - 1, sts.pop(s - 1)) if s > 0 else ()
            attention(s, sts[s], filler)
        for task in norm_tasks(NSEG - 1, sts.pop(NSEG - 1)):
            task()

    nc.compile()
    return nc


def make_in_maps(x, Wqkv, bqkv, Wout, bout):
    """Shard full inputs across 8 cores: core = o*B + b."""
    r, E3, D = Wqkv.shape
    Bb, S, _ = x.shape
    DC = D // P
    in_maps = []
    for c in range(r * Bb):
        o, b = c // Bb, c % Bb
        in_maps.append({
            "xoT": np.ascontiguousarray(x[b, o::r, :].T).astype(ml_dtypes.bfloat16),
            "wqkvT": np.ascontiguousarray(Wqkv[o].T).astype(ml_dtypes.bfloat16),
            "woutT": np.ascontiguousarray(Wout[o].T).astype(ml_dtypes.bfloat16),
            "bqkv_pt": np.ascontiguousarray(bqkv[o].reshape(3 * DC, P).T),
            "bout_pt": np.ascontiguousarray(bout[o].reshape(DC, P).T),
            "bv": np.ascontiguousarray(bqkv[o, 2 * D:3 * D]),
        })
    return in_maps


_NC_CACHE = {}


def get_nc():
    if "nc" not in _NC_CACHE:
        _NC_CACHE["nc"] = build_nc()
    return _NC_CACHE["nc"]


def run(inputs, trace=False, **kwargs):
    """Run the SPMD kernel; returns (full_output, BassKernelResults)."""
    x = np.ascontiguousarray(np.asarray(inputs["x"], dtype=np.float32))
    Wqkv = np.asarray(inputs["Wqkv"], dtype=np.float32)
    bqkv = np.asarray(inputs["bqkv"], dtype=np.float32)
    Wout = np.asarray(inputs["Wout"], dtype=np.float32)
    bout = np.asarray(inputs["bout"], dtype=np.float32)
    r, E3, D = Wqkv.shape
    Bb, S, _ = x.shape

    nc = get_nc()
    in_maps = make_in_maps(x, Wqkv, bqkv, Wout, bout)
    res = run_bass_kernel_spmd(nc, in_maps, core_ids=list(range(len(in_maps))),
                               trace=trace, **kwargs)

    out = np.zeros((Bb, S, r * D), np.float32)
    for c in range(len(in_maps)):
        o, b = c // Bb, c % Bb
        out[b, o::r, o * D:(o + 1) * D] = res.results[c]["outT"].T
    return out, res


def kernel(x, Wqkv, bqkv, Wout, bout, num_heads):
    assert int(num_heads) == H0
    out, _ = run(dict(x=x, Wqkv=Wqkv, bqkv=bqkv, Wout=Wout, bout=bout))
    return out
